# revision 18
# baseline (speedup 1.0000x reference)
"""Distributed exact k-NN retrieval (scores + top-k + gather) on 8 Trainium2
NeuronCores.

Strategy (standard distributed ANN): mat is sharded row-wise across the 8
cores. Each core streams its 64 MB shard once from HBM, computes the f32
scores mat_shard @ query with fused DVE multiply-accumulate ops, then uses
the DVE Max8 / MaxIndex8 instructions to produce its per-partition top-8
candidates (128*8 = 1024 per core) and gathers the rows of the top
NCAND_ROWS=4 of them with indirect DMAs. The host merges the 8 * 1024
candidates down to the global top-k.

Device-side shard layout: rows are permuted host-side so partition p holds
rows p*496 .. p*496+495 of the (padded) shard, in order. Free-dim position
j of the score buffer then directly addresses shard row p*496 + j, so the
indirect-gather offsets need only one integer add (exact even in the DVE's
fp32 ALUs) instead of bit arithmetic.

Candidate coverage: per-partition top-8 covers the global top-64 as long as
no (core, partition) bin holds more than 8 of the global top-64 rows; rows
spread over 1024 bins, so P[any bin >= 9] ~ 1e-14 (actual dataset max: 2).
Rows are gathered on-device only for the top 4 ranks per partition (same
argument, P[any bin >= 5] ~ 1e-9); if a selected candidate ever exceeds
rank 4, its row is filled from host memory in the unshard step.
"""
import numpy as np

import concourse.bacc as bacc
import concourse.bass as bass
import concourse.mybir as mybir
import concourse.tile as tile
from concourse.bass_utils import run_bass_kernel_spmd

P = 128                 # SBUF partitions
D = 256                 # feature dim
NCORES = 8
SHARD = 62500           # real rows per core
JCOLS = 496             # score columns = rows per partition (padded)
SHARD_PAD = P * JCOLS   # 63488 (zero-padded; pad scores are ~0, never top-8)
NCAND = 8               # candidates per partition (Max8)
NCAND_ROWS = 4          # ranks per partition whose rows are gathered on-device

# defaults (tuned)
CH = 16                 # rows per partition per DMA tile; must divide JCOLS
MAT_BUFS = 6
ALT_RINGS = True

_F32 = mybir.dt.float32
_U32 = mybir.dt.uint32


def _build_module(repeat=1, mat_bufs=MAT_BUFS, ch=CH, alt_rings=ALT_RINGS):
    assert JCOLS % ch == 0
    nt = JCOLS // ch
    nc = bacc.Bacc("TRN2", target_bir_lowering=False, debug=False,
                   enable_asserts=False)
    mat = nc.dram_tensor("mat", [SHARD_PAD, D], _F32, kind="ExternalInput").ap()
    qw = nc.dram_tensor("qw", [P, D], _F32, kind="ExternalInput").ap()
    vals8 = nc.dram_tensor("vals8", [P, NCAND], _F32, kind="ExternalOutput").ap()
    idx8 = nc.dram_tensor("idx8", [P, NCAND], _U32, kind="ExternalOutput").ap()
    rowsg = nc.dram_tensor("rowsg", [P, NCAND_ROWS * D], _F32,
                           kind="ExternalOutput").ap()

    # device rows are permuted: dev row p*JCOLS + t*ch + c
    mat_r = mat.rearrange("(p t c) d -> t p (c d)", p=P, c=ch)

    with tile.TileContext(nc) as tc, \
            tc.tile_pool(name="const", bufs=1) as const_pool, \
            tc.tile_pool(name="mat", bufs=mat_bufs) as mat_pool, \
            tc.tile_pool(name="scr", bufs=4) as scr_pool, \
            tc.tile_pool(name="misc", bufs=1) as misc_pool:

        q_sb = const_pool.tile([P, D], _F32)
        nc.sync.dma_start(q_sb[:], qw[:])

        pp = misc_pool.tile([P, NCAND], _U32, tag="pp")
        nc.gpsimd.iota(pp[:], pattern=[[0, NCAND]], base=0,
                       channel_multiplier=JCOLS)

        def body():
            scores = misc_pool.tile([P, JCOLS], _F32, tag="scores")

            def chunk_ops(mt, ch_eff, col0):
                # fused multiply + free-dim sum on DVE, one op per row-chunk
                for c in range(ch_eff):
                    scr = scr_pool.tile([P, D], _F32, tag="fscr")
                    nc.vector.scalar_tensor_tensor(
                        out=scr[:],
                        in0=mt[:, c * D:(c + 1) * D],
                        scalar=0.0,
                        in1=q_sb[:],
                        op0=mybir.AluOpType.bypass,
                        op1=mybir.AluOpType.mult,
                        accum_out=scores[:, col0 + c: col0 + c + 1],
                    )

            for t in range(nt):
                mt = mat_pool.tile([P, ch * D], _F32, tag="mt")
                # alternate between the two HWDGE rings (SP / ACT) so the
                # per-DMA fixed issue cost overlaps across rings
                eng = nc.scalar if (alt_rings and t % 2) else nc.sync
                eng.dma_start(mt[:], mat_r[t])
                chunk_ops(mt, ch, t * ch)

            # per-partition top-8 (values descending) + free-dim positions
            v8 = misc_pool.tile([P, NCAND], _F32, tag="v8")
            i8 = misc_pool.tile([P, NCAND], _U32, tag="i8")
            nc.vector.max(out=v8[:], in_=scores[:])
            nc.vector.max_index(out=i8[:], in_max=v8[:], in_values=scores[:])
            nc.sync.dma_start(vals8[:], v8[:])
            nc.sync.dma_start(idx8[:], i8[:])

            # shard-row ids: r = p*JCOLS + j  (single exact integer add)
            r = misc_pool.tile([P, NCAND], _U32, tag="r")
            nc.vector.tensor_tensor(r[:], i8[:], pp[:], mybir.AluOpType.add)

            # gather rows of the top NCAND_ROWS candidates per partition.
            # One call per rank: the HW DGE only honors per-partition
            # indirect offsets with a single offset per partition ([P, 1]).
            rows_sb = misc_pool.tile([P, NCAND_ROWS * D], _F32, tag="rows")
            for jj in range(NCAND_ROWS):
                nc.gpsimd.indirect_dma_start(
                    out=rows_sb[:, jj * D:(jj + 1) * D],
                    out_offset=None,
                    in_=mat[:],
                    in_offset=bass.IndirectOffsetOnAxis(ap=r[:, jj:jj + 1], axis=0),
                )
            nc.sync.dma_start(rowsg[:], rows_sb[:])

        if repeat == 1:
            body()
        else:
            with tc.For_i(0, repeat, 1):
                body()

    nc.compile()
    return nc


_NC_CACHE = {}


def _get_module(**kw):
    key = tuple(sorted(kw.items()))
    if key not in _NC_CACHE:
        _NC_CACHE[key] = _build_module(**kw)
    return _NC_CACHE[key]


def _prep_shards(mat, ch=CH):
    """Pad to SHARD_PAD rows/core and permute into the device layout:
    dev[core, p*JCOLS + t*ch + c] = orig[core, t*(P*ch) + p*ch + c]."""
    nt = JCOLS // ch
    padded = np.zeros((NCORES, SHARD_PAD, D), dtype=np.float32)
    padded[:, :SHARD] = np.asarray(mat, np.float32).reshape(NCORES, SHARD, D)
    dev = padded.reshape(NCORES, nt, P, ch, D).transpose(0, 2, 1, 3, 4)
    return np.ascontiguousarray(dev.reshape(NCORES, SHARD_PAD, D))


def _make_qw(query, ch=CH):
    return np.ascontiguousarray(np.tile(np.asarray(query, np.float32), (P, 1)))


def _run_device(shards, qw, trace=False, repeat=1, **kw):
    nc = _get_module(repeat=repeat, **kw)
    in_maps = [{"mat": shards[i], "qw": qw} for i in range(NCORES)]
    return run_bass_kernel_spmd(nc, in_maps, core_ids=list(range(NCORES)),
                                trace=trace)


def _merge(res, k, mat, ch=CH):
    vals = np.stack([r["vals8"] for r in res.results])            # [8,128,8] f32
    idxs = np.stack([r["idx8"] for r in res.results])             # [8,128,8] u32
    rows = np.stack([r["rowsg"] for r in res.results])            # [8,128,NR*D]
    rows = rows.reshape(NCORES * P * NCAND_ROWS, D)

    j = idxs.astype(np.int64)                                     # j = t*ch + c
    p_term = (np.arange(P, dtype=np.int64) * ch)[None, :, None]
    r_orig = (j // ch) * (P * ch) + p_term + (j % ch)             # original row
    g = r_orig + (np.arange(NCORES, dtype=np.int64) * SHARD)[:, None, None]

    # pointer into the gathered-row buffer for rank < NCAND_ROWS, else -1
    cp = (np.arange(NCORES * P, dtype=np.int64) * NCAND_ROWS).reshape(NCORES, P, 1)
    rank = np.broadcast_to(np.arange(NCAND, dtype=np.int64), (NCORES, P, NCAND))
    rowptr = np.where(rank < NCAND_ROWS, cp + rank, -1)

    vals_f = vals.reshape(-1)
    g_f = g.reshape(-1)
    rp_f = rowptr.reshape(-1)
    valid = r_orig.reshape(-1) < SHARD
    vals_f, g_f, rp_f = vals_f[valid], g_f[valid], rp_f[valid]

    # jax.lax.top_k order: by value desc, ties -> lower index first
    sel = np.lexsort((g_f, -vals_f))[:k]
    idx = g_f[sel].astype(np.int32)
    out = np.empty((k, D), dtype=np.float32)
    rp_sel = rp_f[sel]
    have = rp_sel >= 0
    out[have] = rows[rp_sel[have]]
    if not np.all(have):
        # astronomically unlikely (a (core,partition) bin held >4 of the
        # global top-k): fill the missing rows from host memory
        out[~have] = np.asarray(mat, np.float32)[idx[~have]]
    return out, idx


def kernel(query, mat, k):
    k = int(k)
    shards = _prep_shards(mat)
    qw = _make_qw(query)
    res = _run_device(shards, qw)
    return _merge(res, k, mat)


# revision 20
# speedup vs baseline: 1.1901x; 1.1901x over previous
"""Distributed exact k-NN retrieval (scores + top-k + gather) on 8 Trainium2
NeuronCores.

Strategy (standard distributed ANN): mat is sharded row-wise across the 8
cores. Each core streams its 64 MB shard once from HBM, computes the f32
scores mat_shard @ query with fused DVE multiply-accumulate ops, then uses
the DVE Max8 / MaxIndex8 instructions to produce its per-partition top-8
candidates (128*8 = 1024 per core) and gathers the rows of the top
NCAND_ROWS of them with indirect DMAs. The host merges the 8 * 1024
candidates down to the global top-k.

Device-side shard layout: rows are permuted host-side so partition p holds
rows p*496 .. p*496+495 of the (padded) shard, in order. Free-dim position
j of the score buffer then directly addresses shard row p*496 + j, so the
indirect-gather offsets need only one integer add (exact even in the DVE's
fp32 ALUs) instead of bit arithmetic.

Candidate coverage: per-partition top-8 covers the global top-64 as long as
no (core, partition) bin holds more than 8 of the global top-64 rows; rows
spread over 1024 bins, so P[any bin >= 9] ~ 1e-14 (actual dataset max: 2).
Rows are gathered on-device only for the top 2 ranks per partition (the
actual dataset's max bin load is 2); if a selected candidate ever exceeds
rank 2, its row is filled from host memory in the unshard step, so
correctness never depends on this bound.
"""
import numpy as np

import concourse.bacc as bacc
import concourse.bass as bass
import concourse.mybir as mybir
import concourse.tile as tile
from concourse.bass_utils import run_bass_kernel_spmd

P = 128                 # SBUF partitions
D = 256                 # feature dim
NCORES = 8
SHARD = 62500           # real rows per core
JCOLS = 496             # score columns = rows per partition (padded)
SHARD_PAD = P * JCOLS   # 63488 (zero-padded; pad scores are ~0, never top-8)
NCAND = 8               # candidates per partition (Max8)
NCAND_ROWS = 2          # ranks per partition whose rows are gathered on-device

# defaults (tuned)
CH = 16                 # rows per partition per DMA tile; must divide JCOLS
MAT_BUFS = 6
ALT_RINGS = True

_F32 = mybir.dt.float32
_U32 = mybir.dt.uint32


def _build_module(repeat=1, mat_bufs=MAT_BUFS, ch=CH, alt_rings=ALT_RINGS):
    assert JCOLS % ch == 0
    nt = JCOLS // ch
    nc = bacc.Bacc("TRN2", target_bir_lowering=False, debug=False,
                   enable_asserts=False)
    mat = nc.dram_tensor("mat", [SHARD_PAD, D], _F32, kind="ExternalInput").ap()
    qw = nc.dram_tensor("qw", [P, D], _F32, kind="ExternalInput").ap()
    vals8 = nc.dram_tensor("vals8", [P, NCAND], _F32, kind="ExternalOutput").ap()
    idx8 = nc.dram_tensor("idx8", [P, NCAND], _U32, kind="ExternalOutput").ap()
    rowsg = nc.dram_tensor("rowsg", [P, NCAND_ROWS * D], _F32,
                           kind="ExternalOutput").ap()

    # device rows are permuted: dev row p*JCOLS + t*ch + c
    mat_r = mat.rearrange("(p t c) d -> t p (c d)", p=P, c=ch)

    with tile.TileContext(nc) as tc, \
            tc.tile_pool(name="const", bufs=1) as const_pool, \
            tc.tile_pool(name="mat", bufs=mat_bufs) as mat_pool, \
            tc.tile_pool(name="scr", bufs=4) as scr_pool, \
            tc.tile_pool(name="misc", bufs=1) as misc_pool:

        q_sb = const_pool.tile([P, D], _F32)
        nc.sync.dma_start(q_sb[:], qw[:])

        pp = misc_pool.tile([P, NCAND], _U32, tag="pp")
        nc.gpsimd.iota(pp[:], pattern=[[0, NCAND]], base=0,
                       channel_multiplier=JCOLS)

        def body():
            scores = misc_pool.tile([P, JCOLS], _F32, tag="scores")

            def chunk_ops(mt, ch_eff, col0):
                # fused multiply + free-dim sum on DVE, one op per row-chunk
                for c in range(ch_eff):
                    scr = scr_pool.tile([P, D], _F32, tag="fscr")
                    nc.vector.scalar_tensor_tensor(
                        out=scr[:],
                        in0=mt[:, c * D:(c + 1) * D],
                        scalar=0.0,
                        in1=q_sb[:],
                        op0=mybir.AluOpType.bypass,
                        op1=mybir.AluOpType.mult,
                        accum_out=scores[:, col0 + c: col0 + c + 1],
                    )

            for t in range(nt):
                mt = mat_pool.tile([P, ch * D], _F32, tag="mt")
                # alternate between the two HWDGE rings (SP / ACT) so the
                # per-DMA fixed issue cost overlaps across rings
                eng = nc.scalar if (alt_rings and t % 2) else nc.sync
                eng.dma_start(mt[:], mat_r[t])
                chunk_ops(mt, ch, t * ch)

            # per-partition top-8 (values descending) + free-dim positions
            v8 = misc_pool.tile([P, NCAND], _F32, tag="v8")
            i8 = misc_pool.tile([P, NCAND], _U32, tag="i8")
            nc.vector.max(out=v8[:], in_=scores[:])
            nc.vector.max_index(out=i8[:], in_max=v8[:], in_values=scores[:])
            nc.sync.dma_start(vals8[:], v8[:])
            nc.sync.dma_start(idx8[:], i8[:])

            # shard-row ids: r = p*JCOLS + j  (single exact integer add)
            r = misc_pool.tile([P, NCAND], _U32, tag="r")
            nc.vector.tensor_tensor(r[:], i8[:], pp[:], mybir.AluOpType.add)

            # gather rows of the top NCAND_ROWS candidates per partition.
            # One call per rank: the HW DGE only honors per-partition
            # indirect offsets with a single offset per partition ([P, 1]).
            rows_sb = misc_pool.tile([P, NCAND_ROWS * D], _F32, tag="rows")
            for jj in range(NCAND_ROWS):
                nc.gpsimd.indirect_dma_start(
                    out=rows_sb[:, jj * D:(jj + 1) * D],
                    out_offset=None,
                    in_=mat[:],
                    in_offset=bass.IndirectOffsetOnAxis(ap=r[:, jj:jj + 1], axis=0),
                )
            nc.sync.dma_start(rowsg[:], rows_sb[:])

        if repeat == 1:
            body()
        else:
            with tc.For_i(0, repeat, 1):
                body()

    nc.compile()
    return nc


_NC_CACHE = {}


def _get_module(**kw):
    key = tuple(sorted(kw.items()))
    if key not in _NC_CACHE:
        _NC_CACHE[key] = _build_module(**kw)
    return _NC_CACHE[key]


def _prep_shards(mat, ch=CH):
    """Pad to SHARD_PAD rows/core and permute into the device layout:
    dev[core, p*JCOLS + t*ch + c] = orig[core, t*(P*ch) + p*ch + c]."""
    nt = JCOLS // ch
    padded = np.zeros((NCORES, SHARD_PAD, D), dtype=np.float32)
    padded[:, :SHARD] = np.asarray(mat, np.float32).reshape(NCORES, SHARD, D)
    dev = padded.reshape(NCORES, nt, P, ch, D).transpose(0, 2, 1, 3, 4)
    return np.ascontiguousarray(dev.reshape(NCORES, SHARD_PAD, D))


def _make_qw(query, ch=CH):
    return np.ascontiguousarray(np.tile(np.asarray(query, np.float32), (P, 1)))


def _run_device(shards, qw, trace=False, repeat=1, **kw):
    nc = _get_module(repeat=repeat, **kw)
    in_maps = [{"mat": shards[i], "qw": qw} for i in range(NCORES)]
    return run_bass_kernel_spmd(nc, in_maps, core_ids=list(range(NCORES)),
                                trace=trace)


def _merge(res, k, mat, ch=CH):
    vals = np.stack([r["vals8"] for r in res.results])            # [8,128,8] f32
    idxs = np.stack([r["idx8"] for r in res.results])             # [8,128,8] u32
    rows = np.stack([r["rowsg"] for r in res.results])            # [8,128,NR*D]
    rows = rows.reshape(NCORES * P * NCAND_ROWS, D)

    j = idxs.astype(np.int64)                                     # j = t*ch + c
    p_term = (np.arange(P, dtype=np.int64) * ch)[None, :, None]
    r_orig = (j // ch) * (P * ch) + p_term + (j % ch)             # original row
    g = r_orig + (np.arange(NCORES, dtype=np.int64) * SHARD)[:, None, None]

    # pointer into the gathered-row buffer for rank < NCAND_ROWS, else -1
    cp = (np.arange(NCORES * P, dtype=np.int64) * NCAND_ROWS).reshape(NCORES, P, 1)
    rank = np.broadcast_to(np.arange(NCAND, dtype=np.int64), (NCORES, P, NCAND))
    rowptr = np.where(rank < NCAND_ROWS, cp + rank, -1)

    vals_f = vals.reshape(-1)
    g_f = g.reshape(-1)
    rp_f = rowptr.reshape(-1)
    valid = r_orig.reshape(-1) < SHARD
    vals_f, g_f, rp_f = vals_f[valid], g_f[valid], rp_f[valid]

    # jax.lax.top_k order: by value desc, ties -> lower index first
    sel = np.lexsort((g_f, -vals_f))[:k]
    idx = g_f[sel].astype(np.int32)
    out = np.empty((k, D), dtype=np.float32)
    rp_sel = rp_f[sel]
    have = rp_sel >= 0
    out[have] = rows[rp_sel[have]]
    if not np.all(have):
        # astronomically unlikely (a (core,partition) bin held >4 of the
        # global top-k): fill the missing rows from host memory
        out[~have] = np.asarray(mat, np.float32)[idx[~have]]
    return out, idx


def kernel(query, mat, k):
    k = int(k)
    shards = _prep_shards(mat)
    qw = _make_qw(query)
    res = _run_device(shards, qw)
    return _merge(res, k, mat)
